# revision 33
# baseline (speedup 1.0000x reference)
"""Trainium2 Bass kernel for a full GPT block (LN -> QKV+RoPE -> full attention
-> out-proj -> residual -> LN -> GELU MLP -> residual).

Sharding: 8 cores = (batch b = core//2) x (query-half = core%2). Each core
redundantly computes K/V for its batch's full 2048 tokens (zero collectives);
Q/out-proj/MLP only for its own 1024 tokens. Tokens are ROTATED per core on
the host so the core's query half is always tokens [0, TQ) — attention is
permutation-invariant over keys, and RoPE tables are rotated to match.

Layout: activations live transposed [d_model on partitions, tokens on free].
LayerNorm stats are computed with ones-matmuls over the partition axis
(bf16, full PE rate) and broadcast back via a rank-1 PE matmul — no PE
transposes anywhere. The final output is DMA'd transposed and fixed on host.

Precision: weights/activations bf16 except (a) q/k projections run fp8e4
DoubleRow (softmax normalization washes out score noise), (b) PV runs fp8
DoubleRow (exp output + V cast to fp8), (c) optionally fc1 fp8. PSUM is f32
throughout; residual stream is f32.
"""

import sys

sys.path.insert(0, "/opt/trn_rl_repo")

from contextlib import ExitStack

import ml_dtypes
import numpy as np

import concourse.bass as bass  # noqa: F401
from concourse import bacc
import concourse.mybir as mybir
import concourse.tile as tile
from concourse.bass_utils import run_bass_kernel_spmd

B, T, D = 4, 2048, 512
H = 8
HD = 64
HALF = 32
EPS = 1e-5
TQ = T // 2          # tokens per core (query half)
DFF = 4 * D          # 2048
NC = 8
WS = 16.0            # fp8 weight pre-scale (power of 2, exact)

F32 = mybir.dt.float32
BF16 = mybir.dt.bfloat16
F8 = mybir.dt.float8e4
AF = mybir.ActivationFunctionType
ALU = mybir.AluOpType
DR = mybir.MatmulPerfMode.DoubleRow

FC1_FP8 = True       # fc1 (h2 @ w1) in fp8 DoubleRow
FC2_FP8 = True       # fc2 (a @ w2) in fp8 DoubleRow
V_FP8 = True         # v projection in fp8 DoubleRow (drops the bf16 LN copy)
DEBUG = False        # add intermediate DRAM outputs for stage-by-stage check

_CACHE = {}


def _rope_tables():
    inv_freq = 1.0 / (10000.0 ** (np.arange(HALF, dtype=np.float64) / HALF))
    angle = np.arange(T, dtype=np.float64)[:, None] * inv_freq[None, :]
    cos = np.cos(angle).astype(np.float32)   # (T, 32)
    sin = np.sin(angle).astype(np.float32)
    # replicated to the 128-partition layout of q^T/k^T tiles:
    # rows [0:32]=head_even half1, [32:64]=head_even half2, [64:96]=head_odd
    # h1, [96:128]=head_odd h2. cos repeats every 32 rows; the swap-multiplier
    # sign is -sin for half1 rows and +sin for half2 rows.
    crep = np.tile(cos.T, (4, 1))                                  # (128, T)
    srep = np.concatenate([-sin.T, sin.T, -sin.T, sin.T], axis=0)  # (128, T)
    return crep.astype(ml_dtypes.bfloat16), srep.astype(ml_dtypes.bfloat16)


def _build_program():
    h2_dt = F8 if FC1_FP8 else BF16
    a_dt = F8 if FC2_FP8 else BF16
    w1_dt = F8 if FC1_FP8 else BF16
    w2_dt = F8 if FC2_FP8 else BF16

    nc = bacc.Bacc("TRN2", target_bir_lowering=False)
    dp = nc.declare_dram_parameter
    d_xt = dp("xt", [D, T], BF16, isOutput=False)       # x^T, rotated
    d_wqk = dp("wqk", [D, 2 * D], F8, isOutput=False)   # x WS, ln1_g folded
    d_wv = dp("wv", [D, D], F8 if V_FP8 else BF16, isOutput=False)
    d_wout = dp("wout", [D, D], F8, isOutput=False)     # x WS
    d_w1 = dp("w1", [D, DFF], w1_dt, isOutput=False)    # (x WS) ln2_g folded
    d_w2 = dp("w2", [DFF, D], w2_dt, isOutput=False)    # (x WS)
    d_b1 = dp("b1t", [128, DFF // 128], F32, isOutput=False)
    d_ck = dp("cos_k", [128, T], BF16, isOutput=False)  # rotated
    d_sk = dp("sin_k", [128, T], BF16, isOutput=False)
    d_out = dp("out", [D, TQ], F32, isOutput=True)      # transposed out
    if DEBUG:
        d_dbg = {
            "nT8": dp("dbg_nT8", [128, 4, T], F8, isOutput=True),
            "nTb": dp("dbg_nTb", [128, 4, T], BF16, isOutput=True),
            "qT": dp("dbg_qT", [128, 4, TQ], F8, isOutput=True),
            "kT": dp("dbg_kT", [128, 4, T], F8, isOutput=True),
            "vnat": dp("dbg_vnat", [128, 16, 8, 66], F8, isOutput=True),
            "attnT": dp("dbg_attnT", [128, 4, TQ], F8, isOutput=True),
            "r1": dp("dbg_r1", [128, 4, TQ], F32, isOutput=True),
            "h2": dp("dbg_h2", [128, 4, TQ],
                     F8 if FC1_FP8 else BF16, isOutput=True),
        }

    ctx = ExitStack()
    with ctx:
        tc = ctx.enter_context(tile.TileContext(nc))
        # ---------------- persistent pools ----------------
        pw = ctx.enter_context(tc.tile_pool(name="weights", bufs=1))
        pact = ctx.enter_context(tc.tile_pool(name="acts", bufs=1))
        pconst = ctx.enter_context(tc.tile_pool(name="consts", bufs=1))

        xt = pact.tile([128, 4, T], BF16)
        for c in range(4):
            nc.sync.dma_start(xt[:, c, 0:512], d_xt[c * 128:(c + 1) * 128, 0:512])
        for c in range(4):
            nc.sync.dma_start(xt[:, c, 512:T], d_xt[c * 128:(c + 1) * 128, 512:T])

        wqk = pw.tile([128, 4, 2 * D], F8)
        wv = pw.tile([128, 4, D], F8 if V_FP8 else BF16)
        wout = pw.tile([128, 4, D], F8)
        w1 = pw.tile([128, 4, DFF], w1_dt)
        w2 = pw.tile([128, 16, D], w2_dt)
        b1t = pw.tile([128, DFF // 128], F32)
        for c in range(4):
            nc.sync.dma_start(wqk[:, c], d_wqk[c * 128:(c + 1) * 128, :])
        cos_k = pconst.tile([128, T], BF16)
        sin_k = pconst.tile([128, T], BF16)
        nc.sync.dma_start(cos_k[:], d_ck[:])
        nc.sync.dma_start(sin_k[:], d_sk[:])
        for c in range(4):
            nc.sync.dma_start(wv[:, c], d_wv[c * 128:(c + 1) * 128, :])
            nc.sync.dma_start(wout[:, c], d_wout[c * 128:(c + 1) * 128, :])
            nc.sync.dma_start(w1[:, c], d_w1[c * 128:(c + 1) * 128, :])
        for c in range(16):
            nc.sync.dma_start(w2[:, c], d_w2[c * 128:(c + 1) * 128, :])
        nc.sync.dma_start(b1t[:], d_b1[:, :])

        ones_bf = pconst.tile([128, 128], BF16)
        nc.gpsimd.memset(ones_bf[:], 1.0)
        inv_d = pconst.tile([128, 1], BF16)
        nc.gpsimd.memset(inv_d[:], 1.0 / D)
        eps_t = pconst.tile([128, 1], F32)
        nc.gpsimd.memset(eps_t[:], EPS)

        # persistent activations
        nT8 = pact.tile([128, 4, T], F8)           # LN1(x)^T fp8 (q/k GEMMs)
        nTb = None if V_FP8 else pact.tile([128, 4, T], BF16)  # for v GEMM
        kT = pact.tile([128, 4, T], F8)            # k^T (RoPE'd, x WS)
        qT = pact.tile([128, 4, TQ], F8)           # q^T (RoPE'd, x WS)
        # v natural, per-head blocks of 65 cols: 64 v dims + a trailing ones
        # column so the PV matmul also produces the softmax rowsum (row 64)
        vnat = pact.tile([128, 16, 8, 66], F8)
        nc.gpsimd.memset(vnat[:, :, :, 64:65], 1.0)
        nc.gpsimd.memset(vnat[:, :, :, 65:66], 0.0)
        attnT = pact.tile([128, 4, TQ], F8)        # normalized attn out ^T
        r1 = pact.tile([128, 4, TQ], F32)          # x + attn_out  (^T)
        r1b = pact.tile([128, 4, TQ], BF16)        # bf16 copy for LN2 stats
        h2 = pact.tile([128, 4, TQ], h2_dt)        # LN2 out

        def ln_transposed(src8, n_chunks, off, ps_stat, ps_bc, pool,
                          apply_srcs, outs):
            """LayerNorm over the partition (d_model) axis of a transposed
            activation. src8: [128, 4, *] fp8 tile for stats (DoubleRow
            ones-matmuls at 2^-6 weight; the 2^-3 残り is folded into the
            psum->sbuf copy). apply_srcs: per-c tiles read in the apply step;
            outs: destinations written as (apply_src - mu) * rstd."""
            for i in range(off, off + n_chunks):
                cs = slice(i * 512, (i + 1) * 512)
                # mu on row 0 and E[x^2] on row 32 of one PSUM bank
                ps_st = ps_stat.tile([33, 512], F32, tag="ps_st")
                for c in range(4):
                    nc.tensor.matmul(ps_st[0:1], inv_d[:, 0:1],
                                     src8[:, c, cs],
                                     start=(c == 0), stop=(c == 3))
                sq = pool.tile([128, 4, 512], BF16, tag="ln_sq")
                for c in range(4):
                    nc.vector.tensor_mul(sq[:, c], src8[:, c, cs],
                                         src8[:, c, cs])
                for c in range(4):
                    nc.tensor.matmul(ps_st[32:33], inv_d[:, 0:1], sq[:, c],
                                     start=(c == 0), stop=(c == 3))
                mu_r = pool.tile([1, 512], BF16, tag="mu_r")
                nc.vector.tensor_copy(mu_r[:], ps_st[0:1])
                ex2_r = pool.tile([1, 512], F32, tag="ex2_r")
                nc.vector.tensor_copy(ex2_r[:], ps_st[32:33])
                # var = E[x^2] - mu^2 on the narrow stat rows
                m2_r = pool.tile([1, 512], F32, tag="m2_r")
                nc.vector.tensor_mul(m2_r[:], mu_r[:], mu_r[:])
                var_r = pool.tile([1, 512], BF16, tag="var_r")
                nc.vector.tensor_sub(var_r[:], ex2_r[:], m2_r[:])
                bc = ps_bc.tile([128, 2, 512], F32, tag="ps_bc")
                nc.tensor.matmul(bc[:, 0], ones_bf[0:1, :], mu_r[:],
                                 start=True, stop=True)
                nc.tensor.matmul(bc[:, 1], ones_bf[0:1, :], var_r[:],
                                 start=True, stop=True)
                brs = pool.tile([128, 512], F32, tag="ln_brs")
                nc.scalar.activation(brs[:], bc[:, 1], AF.Abs_reciprocal_sqrt,
                                     bias=eps_t[:], scale=1.0)
                for c in range(4):
                    t_sub = pool.tile([128, 512], BF16, tag="ln_tsub")
                    nc.vector.tensor_sub(t_sub[:], apply_srcs[c][:, cs],
                                         bc[:, 0])
                    for out_t in outs:
                        nc.vector.tensor_mul(out_t[:, c, cs], t_sub[:], brs[:])

        # ============ phase A: LN1 + QKV + RoPE ============
        with tc.tile_pool(name="phA", bufs=2) as pa, \
             tc.tile_pool(name="phArope", bufs=2) as pr, \
             tc.tile_pool(name="psumSt", bufs=2, space="PSUM") as ps_stat, \
             tc.tile_pool(name="psumBc", bufs=2, space="PSUM") as ps_bc, \
             tc.tile_pool(name="psumA", bufs=2, space="PSUM") as psum:
            ln_transposed(xt, T // 512, 0, ps_stat, ps_bc, pa,
                          [xt[:, c] for c in range(4)],
                          [nT8] if V_FP8 else [nT8, nTb])

            # q^T [j,t] over the core's half (fp8 DoubleRow)
            for jb in range(4):
                for tcn in range(TQ // 512):
                    ps = psum.tile([128, 512], F32, tag="ps_mm")
                    for c2 in range(2):
                        nc.tensor.matmul(
                            ps[:],
                            wqk[:, 2 * c2:2 * c2 + 2, jb * 128:(jb + 1) * 128],
                            nT8[:, 2 * c2:2 * c2 + 2, tcn * 512:(tcn + 1) * 512],
                            start=(c2 == 0), stop=(c2 == 1), perf_mode=DR)
                    nc.scalar.copy(qT[:, jb, tcn * 512:(tcn + 1) * 512], ps[:])
            # k^T [j,t] over full T (fp8 DoubleRow)
            for jb in range(4):
                for tcn in range(T // 512):
                    ps = psum.tile([128, 512], F32, tag="ps_mm")
                    for c2 in range(2):
                        nc.tensor.matmul(
                            ps[:],
                            wqk[:, 2 * c2:2 * c2 + 2,
                                512 + jb * 128:512 + (jb + 1) * 128],
                            nT8[:, 2 * c2:2 * c2 + 2, tcn * 512:(tcn + 1) * 512],
                            start=(c2 == 0), stop=(c2 == 1), perf_mode=DR)
                    nc.scalar.copy(kT[:, jb, tcn * 512:(tcn + 1) * 512], ps[:])
            # v natural [t, dv] over full T (lhsT = LN1(x)^T chunks)
            for tb in range(T // 128):
                ps = psum.tile([128, 8, 64], F32, tag="ps_mm")
                if V_FP8:
                    for c2 in range(2):
                        nc.tensor.matmul(
                            ps[:],
                            nT8[:, 2 * c2:2 * c2 + 2, tb * 128:(tb + 1) * 128],
                            wv[:, 2 * c2:2 * c2 + 2, :],
                            start=(c2 == 0), stop=(c2 == 1), perf_mode=DR)
                    nc.scalar.mul(vnat[:, tb, :, 0:64], ps[:], 1.0 / WS)
                else:
                    for c in range(4):
                        nc.tensor.matmul(
                            ps[:], nTb[:, c, tb * 128:(tb + 1) * 128],
                            wv[:, c, :], start=(c == 0), stop=(c == 3))
                    nc.scalar.copy(vnat[:, tb, :, 0:64], ps[:])

            # ---------------- RoPE on q^T and k^T (fp8 in place) -----------
            def rope(tsb, jb, t0, W):
                view = tsb[:, jb, t0:t0 + W]
                cs = slice(t0, t0 + W)
                qsw = pr.tile([128, W], F8, tag="rope_swap")
                nc.sync.dma_start(qsw[0:32], view[32:64])
                nc.sync.dma_start(qsw[32:64], view[0:32])
                nc.sync.dma_start(qsw[64:96], view[96:128])
                nc.sync.dma_start(qsw[96:128], view[64:96])
                m1 = pr.tile([128, W], BF16, tag="rope_m1")
                nc.vector.tensor_mul(m1[:], view, cos_k[:, cs])
                m2 = pr.tile([128, W], BF16, tag="rope_m2")
                nc.vector.tensor_mul(m2[:], qsw[:], sin_k[:, cs])
                nc.vector.tensor_add(view, m1[:], m2[:])

            for jb in range(4):
                for t0 in range(0, TQ, 512):
                    rope(qT, jb, t0, 512)
                for t0 in range(0, T, 512):
                    rope(kT, jb, t0, 512)

        # ============ phase B: attention ============
        with tc.tile_pool(name="phB", bufs=2) as pb, \
             tc.tile_pool(name="phBo", bufs=9) as po_pool, \
             tc.tile_pool(name="phBe", bufs=4) as pbe, \
             tc.tile_pool(name="psumS", bufs=2, space="PSUM") as ps_s, \
             tc.tile_pool(name="psumPV", bufs=2, space="PSUM") as ps_pv, \
             tc.tile_pool(name="psumBc2", bufs=2, space="PSUM") as ps_bc2:
            for qc in range(TQ // 512):
                qs = slice(qc * 512, (qc + 1) * 512)
                # rowsums of heads 0-3 / 4-7 land on partitions {0,32,64,96}
                rsA = pb.tile([128, 512], BF16, tag="rsA")
                rsB = pb.tile([128, 512], BF16, tag="rsB")
                o_ts = []
                for h in range(H):
                    g, par = h // 2, h % 2
                    po = par * 64      # partition offset of this head's rows
                    pv = ps_pv.tile([128, 512], F32, tag="ps_pv")
                    for pair in range(T // 256):
                        ps2 = ps_s.tile([128, 2, 512], F32, tag="ps_score")
                        for hf in range(2):
                            kb = 2 * pair + hf
                            nc.tensor.matmul(
                                ps2[:, hf],
                                kT[po:po + 64, g, kb * 128:(kb + 1) * 128],
                                qT[po:po + 64, g, qs],
                                start=True, stop=True)
                        e2 = pbe.tile([128, 2, 512], F8, tag="E2")
                        nc.scalar.activation(e2[:, 0], ps2[:, 0], AF.Exp,
                                             scale=0.125 / (WS * WS))
                        nc.scalar.activation(e2[:, 1], ps2[:, 1], AF.Exp,
                                             scale=0.125 / (WS * WS))
                        # O^T (rows 0..63) + softmax rowsum (row 64)
                        nc.tensor.matmul(
                            pv[0:66],
                            vnat[:, 2 * pair:2 * pair + 2, h, :],
                            e2[:],
                            start=(pair == 0), stop=(pair == T // 256 - 1),
                            perf_mode=DR)
                    o_t = po_pool.tile([128, 512], BF16, tag="o_t")
                    nc.vector.tensor_copy(o_t[0:65], pv[0:65])
                    rs_t = rsA if h < 4 else rsB
                    rp = 32 * (h % 4)
                    nc.sync.dma_start(rs_t[rp:rp + 1], o_t[64:65])
                    o_ts.append(o_t)
                # batched reciprocal of all 8 rowsums (2 ops), then per-head
                # rank-1 broadcast + normalize
                riA = pb.tile([128, 512], BF16, tag="riA")
                riB = pb.tile([128, 512], BF16, tag="riB")
                rq = pb.tile([128, 2, 512], BF16, tag="rsq")
                nc.vector.tensor_mul(rq[:, 0], rsA[:], rsA[:])
                nc.vector.tensor_mul(rq[:, 1], rsB[:], rsB[:])
                nc.scalar.activation(riA[:], rq[:, 0], AF.Abs_reciprocal_sqrt)
                nc.scalar.activation(riB[:], rq[:, 1], AF.Abs_reciprocal_sqrt)
                for h in range(H):
                    g, par = h // 2, h % 2
                    po = par * 64
                    ri_t = riA if h < 4 else riB
                    rp = 32 * (h % 4)
                    bc = ps_bc2.tile([128, 512], F32, tag="ps_bc")
                    nc.tensor.matmul(
                        bc[0:64],
                        ones_bf[rp:rp + 1, 0:64],
                        ri_t[rp:rp + 1, :],
                        start=True, stop=True, tile_position=(rp, 0))
                    # partition-shifted write is legal (out base 64, 64 rows)
                    nc.vector.tensor_mul(attnT[po:po + 64, g, qs],
                                         o_ts[h][0:64], bc[0:64])

        # ============ phase C: out-proj + residual + LN2 ============
        with tc.tile_pool(name="phC", bufs=2) as pc, \
             tc.tile_pool(name="phDa", bufs=2) as pda, \
             tc.tile_pool(name="psumSt2", bufs=2, space="PSUM") as ps_stat2, \
             tc.tile_pool(name="psumBc3", bufs=2, space="PSUM") as ps_bc3, \
             tc.tile_pool(name="psumC", bufs=2, space="PSUM") as psum:
            gelu_scale = 1.0 / WS if FC1_FP8 else 1.0
            fc2_scale = 1.0 / WS if FC2_FP8 else 1.0
            for qc in range(TQ // 512):
                qs = slice(qc * 512, (qc + 1) * 512)
                # out-proj (fp8 DoubleRow) + residual
                for db in range(4):
                    ps = psum.tile([128, 512], F32, tag="ps_mm")
                    for c2 in range(2):
                        nc.tensor.matmul(
                            ps[:],
                            wout[:, 2 * c2:2 * c2 + 2, db * 128:(db + 1) * 128],
                            attnT[:, 2 * c2:2 * c2 + 2, qs],
                            start=(c2 == 0), stop=(c2 == 1), perf_mode=DR)
                    nc.vector.scalar_tensor_tensor(
                        r1[:, db, qs], ps[:], 1.0 / WS, xt[:, db, qs],
                        op0=ALU.mult, op1=ALU.add)
                    nc.vector.tensor_copy(r1b[:, db, qs], r1[:, db, qs])

                ln_transposed(r1b, 1, qc, ps_stat2, ps_bc3, pc,
                              [r1[:, c] for c in range(4)], [h2])

            # MLP (both qc chunks; LN2 applies overlap the other chunk's mms)
            for qc in range(TQ // 512):
                qs = slice(qc * 512, (qc + 1) * 512)
                a_t = pda.tile([128, 16, 512], a_dt, tag="a_t")
                for jb in range(16):
                    ps = psum.tile([128, 512], F32, tag="ps_mm")
                    if FC1_FP8:
                        for c2 in range(2):
                            nc.tensor.matmul(
                                ps[:],
                                w1[:, 2 * c2:2 * c2 + 2,
                                   jb * 128:(jb + 1) * 128],
                                h2[:, 2 * c2:2 * c2 + 2, qs],
                                start=(c2 == 0), stop=(c2 == 1), perf_mode=DR)
                    else:
                        for c in range(4):
                            nc.tensor.matmul(
                                ps[:], w1[:, c, jb * 128:(jb + 1) * 128],
                                h2[:, c, qs], start=(c == 0), stop=(c == 3))
                    nc.scalar.activation(a_t[:, jb], ps[:], AF.Gelu,
                                         bias=b1t[:, jb:jb + 1],
                                         scale=gelu_scale)
                for db in range(4):
                    ps = psum.tile([128, 512], F32, tag="ps_mm")
                    if FC2_FP8:
                        for j2 in range(8):
                            nc.tensor.matmul(
                                ps[:],
                                w2[:, 2 * j2:2 * j2 + 2,
                                   db * 128:(db + 1) * 128],
                                a_t[:, 2 * j2:2 * j2 + 2],
                                start=(j2 == 0), stop=(j2 == 7), perf_mode=DR)
                    else:
                        for jb in range(16):
                            nc.tensor.matmul(
                                ps[:], w2[:, jb, db * 128:(db + 1) * 128],
                                a_t[:, jb], start=(jb == 0), stop=(jb == 15))
                    outT = pc.tile([128, 512], F32, tag="outT")
                    nc.vector.scalar_tensor_tensor(
                        outT[:], ps[:], fc2_scale, r1[:, db, qs],
                        op0=ALU.mult, op1=ALU.add)
                    nc.sync.dma_start(
                        d_out[db * 128:(db + 1) * 128, qs], outT[:])

        if DEBUG:
            for name, t in [("nT8", nT8), ("nTb", nTb), ("qT", qT),
                            ("kT", kT), ("vnat", vnat), ("attnT", attnT),
                            ("r1", r1), ("h2", h2)]:
                if t is not None:
                    nc.sync.dma_start(d_dbg[name][:], t[:])
    nc.finalize()
    return nc


def kernel(x, ln1_g, ln1_b, w_qkv, w_out, ln2_g, ln2_b, w1, b1, w2, b2):
    x = np.asarray(x, np.float32)
    ln1_g = np.asarray(ln1_g, np.float32); ln1_b = np.asarray(ln1_b, np.float32)
    ln2_g = np.asarray(ln2_g, np.float32); ln2_b = np.asarray(ln2_b, np.float32)
    w_qkv = np.asarray(w_qkv, np.float32); w_out = np.asarray(w_out, np.float32)
    w1 = np.asarray(w1, np.float32); b1 = np.asarray(b1, np.float32)
    w2 = np.asarray(w2, np.float32); b2 = np.asarray(b2, np.float32)

    assert not np.any(ln1_b), "nonzero ln1_b not supported by this kernel"

    # exact-math folds: LN affine params into the adjacent weight matrices
    wqkv_f = ln1_g[:, None] * w_qkv
    w1_f = ln2_g[:, None] * w1
    b1_f = b1 + ln2_b @ w1

    bf = ml_dtypes.bfloat16
    f8 = ml_dtypes.float8_e4m3

    def to_f8(a, scale):
        return np.ascontiguousarray(
            np.clip(a * scale, -240.0, 240.0).astype(f8))

    crep, srep = _rope_tables()

    common = {
        "wqk": to_f8(wqkv_f[:, :2 * D], WS),
        "wv": (to_f8(wqkv_f[:, 2 * D:], WS) if V_FP8
               else np.ascontiguousarray(wqkv_f[:, 2 * D:].astype(bf))),
        "wout": to_f8(w_out, WS),
        "w1": (to_f8(w1_f, WS) if FC1_FP8
               else np.ascontiguousarray(w1_f.astype(bf))),
        "w2": (to_f8(w2, WS) if FC2_FP8
               else np.ascontiguousarray(w2.astype(bf))),
        "b1t": np.ascontiguousarray(
            b1_f.reshape(DFF // 128, 128).T.astype(np.float32)),
    }
    in_maps = []
    for c in range(NC):
        b, half = c // 2, c % 2
        t0 = half * TQ
        rot = np.r_[t0:T, 0:t0]
        m = dict(common)
        m["xt"] = np.ascontiguousarray(x[b].T[:, rot].astype(bf))
        m["cos_k"] = np.ascontiguousarray(crep[:, rot])
        m["sin_k"] = np.ascontiguousarray(srep[:, rot])
        in_maps.append(m)

    if "prog" not in _CACHE:
        _CACHE["prog"] = _build_program()
    nc = _CACHE["prog"]

    _CACHE["in_maps"] = in_maps
    res = run_bass_kernel_spmd(nc, in_maps, core_ids=list(range(NC)))
    out = np.empty((B, T, D), np.float32)
    for c in range(NC):
        b, half = c // 2, c % 2
        out[b, half * TQ:(half + 1) * TQ] = res.results[c]["out"].T
    out += b2[None, None, :]
    return out


# revision 34
# speedup vs baseline: 1.0762x; 1.0762x over previous
"""Trainium2 Bass kernel for a full GPT block (LN -> QKV+RoPE -> full attention
-> out-proj -> residual -> LN -> GELU MLP -> residual).

Sharding: 8 cores = (batch b = core//2) x (query-half = core%2). Each core
redundantly computes K/V for its batch's full 2048 tokens (zero collectives);
Q/out-proj/MLP only for its own 1024 tokens. Tokens are ROTATED per core on
the host so the core's query half is always tokens [0, TQ) — attention is
permutation-invariant over keys, and RoPE tables are rotated to match.

Layout: activations live transposed [d_model on partitions, tokens on free].
LayerNorm stats are computed with ones-matmuls over the partition axis
(bf16, full PE rate) and broadcast back via a rank-1 PE matmul — no PE
transposes anywhere. The final output is DMA'd transposed and fixed on host.

Precision: weights/activations bf16 except (a) q/k projections run fp8e4
DoubleRow (softmax normalization washes out score noise), (b) PV runs fp8
DoubleRow (exp output + V cast to fp8), (c) optionally fc1 fp8. PSUM is f32
throughout; residual stream is f32.
"""

import sys

sys.path.insert(0, "/opt/trn_rl_repo")

from contextlib import ExitStack

import ml_dtypes
import numpy as np

import concourse.bass as bass  # noqa: F401
from concourse import bacc
import concourse.mybir as mybir
import concourse.tile as tile
from concourse.bass_utils import run_bass_kernel_spmd

B, T, D = 4, 2048, 512
H = 8
HD = 64
HALF = 32
EPS = 1e-5
TQ = T // 2          # tokens per core (query half)
DFF = 4 * D          # 2048
NC = 8
WS = 16.0            # fp8 weight pre-scale (power of 2, exact)

F32 = mybir.dt.float32
BF16 = mybir.dt.bfloat16
F8 = mybir.dt.float8e4
AF = mybir.ActivationFunctionType
ALU = mybir.AluOpType
DR = mybir.MatmulPerfMode.DoubleRow

FC1_FP8 = True       # fc1 (h2 @ w1) in fp8 DoubleRow
FC2_FP8 = True       # fc2 (a @ w2) in fp8 DoubleRow
V_FP8 = True         # v projection in fp8 DoubleRow (drops the bf16 LN copy)
DEBUG = False        # add intermediate DRAM outputs for stage-by-stage check

_CACHE = {}


def _rope_tables():
    inv_freq = 1.0 / (10000.0 ** (np.arange(HALF, dtype=np.float64) / HALF))
    angle = np.arange(T, dtype=np.float64)[:, None] * inv_freq[None, :]
    cos = np.cos(angle).astype(np.float32)   # (T, 32)
    sin = np.sin(angle).astype(np.float32)
    # replicated to the 128-partition layout of q^T/k^T tiles:
    # rows [0:32]=head_even half1, [32:64]=head_even half2, [64:96]=head_odd
    # h1, [96:128]=head_odd h2. cos repeats every 32 rows; the swap-multiplier
    # sign is -sin for half1 rows and +sin for half2 rows.
    crep = np.tile(cos.T, (4, 1))                                  # (128, T)
    srep = np.concatenate([-sin.T, sin.T, -sin.T, sin.T], axis=0)  # (128, T)
    return crep.astype(ml_dtypes.bfloat16), srep.astype(ml_dtypes.bfloat16)


def _build_program():
    h2_dt = F8 if FC1_FP8 else BF16
    a_dt = F8 if FC2_FP8 else BF16
    w1_dt = F8 if FC1_FP8 else BF16
    w2_dt = F8 if FC2_FP8 else BF16

    nc = bacc.Bacc("TRN2", target_bir_lowering=False)
    dp = nc.declare_dram_parameter
    d_xt = dp("xt", [D, T], BF16, isOutput=False)       # x^T, rotated
    d_wqk = dp("wqk", [D, 2 * D], F8, isOutput=False)   # x WS, ln1_g folded
    d_wv = dp("wv", [D, D], F8 if V_FP8 else BF16, isOutput=False)
    d_wout = dp("wout", [D, D], F8, isOutput=False)     # x WS
    d_w1 = dp("w1", [D, DFF], w1_dt, isOutput=False)    # (x WS) ln2_g folded
    d_w2 = dp("w2", [DFF, D], w2_dt, isOutput=False)    # (x WS)
    d_b1 = dp("b1t", [128, DFF // 128], F32, isOutput=False)
    d_ck = dp("cos_k", [128, T], BF16, isOutput=False)  # rotated
    d_sk = dp("sin_k", [128, T], BF16, isOutput=False)
    d_out = dp("out", [D, TQ], F32, isOutput=True)      # transposed out
    if DEBUG:
        d_dbg = {
            "nT8": dp("dbg_nT8", [128, 4, T], F8, isOutput=True),
            "nTb": dp("dbg_nTb", [128, 4, T], BF16, isOutput=True),
            "qT": dp("dbg_qT", [128, 4, TQ], F8, isOutput=True),
            "kT": dp("dbg_kT", [128, 4, T], F8, isOutput=True),
            "vnat": dp("dbg_vnat", [128, 16, 8, 66], F8, isOutput=True),
            "attnT": dp("dbg_attnT", [128, 4, TQ], F8, isOutput=True),
            "r1": dp("dbg_r1", [128, 4, TQ], F32, isOutput=True),
            "h2": dp("dbg_h2", [128, 4, TQ],
                     F8 if FC1_FP8 else BF16, isOutput=True),
        }

    ctx = ExitStack()
    with ctx:
        tc = ctx.enter_context(tile.TileContext(nc))
        # ---------------- persistent pools ----------------
        pw = ctx.enter_context(tc.tile_pool(name="weights", bufs=1))
        pact = ctx.enter_context(tc.tile_pool(name="acts", bufs=1))
        pconst = ctx.enter_context(tc.tile_pool(name="consts", bufs=1))

        xt = pact.tile([128, 4, T], BF16)
        for c in range(4):
            nc.sync.dma_start(xt[:, c, 0:512], d_xt[c * 128:(c + 1) * 128, 0:512])
        for c in range(4):
            nc.sync.dma_start(xt[:, c, 512:T], d_xt[c * 128:(c + 1) * 128, 512:T])

        wqk = pw.tile([128, 4, 2 * D], F8)
        wv = pw.tile([128, 4, D], F8 if V_FP8 else BF16)
        wout = pw.tile([128, 4, D], F8)
        w1 = pw.tile([128, 4, DFF], w1_dt)
        w2 = pw.tile([128, 16, D], w2_dt)
        b1t = pw.tile([128, DFF // 128], F32)
        for c in range(4):
            nc.sync.dma_start(wqk[:, c], d_wqk[c * 128:(c + 1) * 128, :])
        cos_k = pconst.tile([128, T], BF16)
        sin_k = pconst.tile([128, T], BF16)
        nc.sync.dma_start(cos_k[:], d_ck[:])
        nc.sync.dma_start(sin_k[:], d_sk[:])
        for c in range(4):
            nc.sync.dma_start(wv[:, c], d_wv[c * 128:(c + 1) * 128, :])
            nc.sync.dma_start(wout[:, c], d_wout[c * 128:(c + 1) * 128, :])
            nc.sync.dma_start(w1[:, c], d_w1[c * 128:(c + 1) * 128, :])
        for c in range(16):
            nc.sync.dma_start(w2[:, c], d_w2[c * 128:(c + 1) * 128, :])
        nc.sync.dma_start(b1t[:], d_b1[:, :])

        ones_bf = pconst.tile([128, 128], BF16)
        nc.gpsimd.memset(ones_bf[:], 1.0)
        inv_d = pconst.tile([128, 1], BF16)
        nc.gpsimd.memset(inv_d[:], 1.0 / D)
        eps_t = pconst.tile([128, 1], F32)
        nc.gpsimd.memset(eps_t[:], EPS)

        # persistent activations
        nT8 = pact.tile([128, 4, T], F8)           # LN1(x)^T fp8 (q/k GEMMs)
        nTb = None if V_FP8 else pact.tile([128, 4, T], BF16)  # for v GEMM
        kT = pact.tile([128, 4, T], F8)            # k^T (RoPE'd, x WS)
        qT = pact.tile([128, 4, TQ], F8)           # q^T (RoPE'd, x WS)
        # v natural, per-head blocks of 65 cols: 64 v dims + a trailing ones
        # column so the PV matmul also produces the softmax rowsum (row 64)
        vnat = pact.tile([128, 16, 8, 66], F8)
        nc.gpsimd.memset(vnat[:, :, :, 64:65], 1.0)
        nc.gpsimd.memset(vnat[:, :, :, 65:66], 0.0)
        attnT = pact.tile([128, 4, TQ], F8)        # normalized attn out ^T
        r1 = pact.tile([128, 4, TQ], F32)          # x + attn_out  (^T)
        r1b = pact.tile([128, 4, TQ], BF16)        # bf16 copy for LN2 stats
        h2 = pact.tile([128, 4, TQ], h2_dt)        # LN2 out

        def ln_transposed(src8, n_chunks, off, ps_stat, ps_bc, pool,
                          apply_srcs, outs):
            """LayerNorm over the partition (d_model) axis of a transposed
            activation. src8: [128, 4, *] fp8 tile for stats (DoubleRow
            ones-matmuls at 2^-6 weight; the 2^-3 残り is folded into the
            psum->sbuf copy). apply_srcs: per-c tiles read in the apply step;
            outs: destinations written as (apply_src - mu) * rstd."""
            for i in range(off, off + n_chunks):
                cs = slice(i * 512, (i + 1) * 512)
                # mu on row 0 and E[x^2] on row 32 of one PSUM bank
                ps_st = ps_stat.tile([33, 512], F32, tag="ps_st")
                for c in range(4):
                    nc.tensor.matmul(ps_st[0:1], inv_d[:, 0:1],
                                     src8[:, c, cs],
                                     start=(c == 0), stop=(c == 3))
                sq = pool.tile([128, 4, 512], BF16, tag="ln_sq")
                for c in range(4):
                    nc.vector.tensor_mul(sq[:, c], src8[:, c, cs],
                                         src8[:, c, cs])
                for c in range(4):
                    nc.tensor.matmul(ps_st[32:33], inv_d[:, 0:1], sq[:, c],
                                     start=(c == 0), stop=(c == 3))
                mu_r = pool.tile([1, 512], BF16, tag="mu_r")
                nc.vector.tensor_copy(mu_r[:], ps_st[0:1])
                ex2_r = pool.tile([1, 512], F32, tag="ex2_r")
                nc.vector.tensor_copy(ex2_r[:], ps_st[32:33])
                # var = E[x^2] - mu^2 on the narrow stat rows
                m2_r = pool.tile([1, 512], F32, tag="m2_r")
                nc.vector.tensor_mul(m2_r[:], mu_r[:], mu_r[:])
                var_r = pool.tile([1, 512], BF16, tag="var_r")
                nc.vector.tensor_sub(var_r[:], ex2_r[:], m2_r[:])
                bc = ps_bc.tile([128, 2, 512], F32, tag="ps_bc")
                nc.tensor.matmul(bc[:, 0], ones_bf[0:1, :], mu_r[:],
                                 start=True, stop=True)
                nc.tensor.matmul(bc[:, 1], ones_bf[0:1, :], var_r[:],
                                 start=True, stop=True)
                brs = pool.tile([128, 512], F32, tag="ln_brs")
                nc.scalar.activation(brs[:], bc[:, 1], AF.Abs_reciprocal_sqrt,
                                     bias=eps_t[:], scale=1.0)
                for c in range(4):
                    t_sub = pool.tile([128, 512], BF16, tag="ln_tsub")
                    nc.vector.tensor_sub(t_sub[:], apply_srcs[c][:, cs],
                                         bc[:, 0])
                    for out_t in outs:
                        nc.vector.tensor_mul(out_t[:, c, cs], t_sub[:], brs[:])

        # ============ phase A: LN1 + QKV + RoPE ============
        with tc.tile_pool(name="phA", bufs=2) as pa, \
             tc.tile_pool(name="phArope", bufs=2) as pr, \
             tc.tile_pool(name="psumSt", bufs=2, space="PSUM") as ps_stat, \
             tc.tile_pool(name="psumBc", bufs=2, space="PSUM") as ps_bc, \
             tc.tile_pool(name="psumA", bufs=2, space="PSUM") as psum:
            ln_transposed(xt, T // 512, 0, ps_stat, ps_bc, pa,
                          [xt[:, c] for c in range(4)],
                          [nT8] if V_FP8 else [nT8, nTb])

            # q^T [j,t] over the core's half (fp8 DoubleRow)
            for jb in range(4):
                for tcn in range(TQ // 512):
                    ps = psum.tile([128, 512], F32, tag="ps_mm")
                    for c2 in range(2):
                        nc.tensor.matmul(
                            ps[:],
                            wqk[:, 2 * c2:2 * c2 + 2, jb * 128:(jb + 1) * 128],
                            nT8[:, 2 * c2:2 * c2 + 2, tcn * 512:(tcn + 1) * 512],
                            start=(c2 == 0), stop=(c2 == 1), perf_mode=DR)
                    nc.scalar.copy(qT[:, jb, tcn * 512:(tcn + 1) * 512], ps[:])
            # k^T [j,t] over full T (fp8 DoubleRow)
            for jb in range(4):
                for tcn in range(T // 512):
                    ps = psum.tile([128, 512], F32, tag="ps_mm")
                    for c2 in range(2):
                        nc.tensor.matmul(
                            ps[:],
                            wqk[:, 2 * c2:2 * c2 + 2,
                                512 + jb * 128:512 + (jb + 1) * 128],
                            nT8[:, 2 * c2:2 * c2 + 2, tcn * 512:(tcn + 1) * 512],
                            start=(c2 == 0), stop=(c2 == 1), perf_mode=DR)
                    nc.scalar.copy(kT[:, jb, tcn * 512:(tcn + 1) * 512], ps[:])
            # v natural [t, dv] over full T (lhsT = LN1(x)^T chunks)
            for tb in range(T // 128):
                ps = psum.tile([128, 8, 64], F32, tag="ps_mm")
                if V_FP8:
                    for c2 in range(2):
                        nc.tensor.matmul(
                            ps[:],
                            nT8[:, 2 * c2:2 * c2 + 2, tb * 128:(tb + 1) * 128],
                            wv[:, 2 * c2:2 * c2 + 2, :],
                            start=(c2 == 0), stop=(c2 == 1), perf_mode=DR)
                    nc.scalar.mul(vnat[:, tb, :, 0:64], ps[:], 1.0 / WS)
                else:
                    for c in range(4):
                        nc.tensor.matmul(
                            ps[:], nTb[:, c, tb * 128:(tb + 1) * 128],
                            wv[:, c, :], start=(c == 0), stop=(c == 3))
                    nc.scalar.copy(vnat[:, tb, :, 0:64], ps[:])

            # ---------------- RoPE on q^T and k^T (fp8 in place) -----------
            def rope(tsb, jb, t0, W):
                view = tsb[:, jb, t0:t0 + W]
                cs = slice(t0, t0 + W)
                qsw = pr.tile([128, W], F8, tag="rope_swap")
                nc.sync.dma_start(qsw[0:32], view[32:64])
                nc.sync.dma_start(qsw[32:64], view[0:32])
                nc.sync.dma_start(qsw[64:96], view[96:128])
                nc.sync.dma_start(qsw[96:128], view[64:96])
                m1 = pr.tile([128, W], BF16, tag="rope_m1")
                nc.vector.tensor_mul(m1[:], view, cos_k[:, cs])
                m2 = pr.tile([128, W], BF16, tag="rope_m2")
                nc.vector.tensor_mul(m2[:], qsw[:], sin_k[:, cs])
                nc.vector.tensor_add(view, m1[:], m2[:])

            for jb in range(4):
                for t0 in range(0, TQ, 512):
                    rope(qT, jb, t0, 512)
                for t0 in range(0, T, 512):
                    rope(kT, jb, t0, 512)

        # ============ phase B: attention ============
        with tc.tile_pool(name="phB", bufs=2) as pb, \
             tc.tile_pool(name="phBo", bufs=9) as po_pool, \
             tc.tile_pool(name="phBe", bufs=4) as pbe, \
             tc.tile_pool(name="psumS", bufs=2, space="PSUM") as ps_s, \
             tc.tile_pool(name="psumPV", bufs=2, space="PSUM") as ps_pv, \
             tc.tile_pool(name="psumBc2", bufs=2, space="PSUM") as ps_bc2:
            for qc in range(TQ // 512):
                qs = slice(qc * 512, (qc + 1) * 512)
                # rowsums of heads 0-3 / 4-7 land on partitions {0,32,64,96}
                rsA = pb.tile([128, 512], BF16, tag="rsA")
                rsB = pb.tile([128, 512], BF16, tag="rsB")
                o_ts = []
                for h in range(H):
                    g, par = h // 2, h % 2
                    po = par * 64      # partition offset of this head's rows
                    pv = ps_pv.tile([128, 512], F32, tag="ps_pv")
                    for pair in range(T // 256):
                        ps2 = ps_s.tile([128, 2, 512], F32, tag="ps_score")
                        for hf in range(2):
                            kb = 2 * pair + hf
                            nc.tensor.matmul(
                                ps2[:, hf],
                                kT[po:po + 64, g, kb * 128:(kb + 1) * 128],
                                qT[po:po + 64, g, qs],
                                start=True, stop=True)
                        e2 = pbe.tile([128, 2, 512], F8, tag="E2")
                        nc.scalar.activation(e2[:], ps2[:], AF.Exp,
                                             scale=0.125 / (WS * WS))
                        # O^T (rows 0..63) + softmax rowsum (row 64)
                        nc.tensor.matmul(
                            pv[0:66],
                            vnat[:, 2 * pair:2 * pair + 2, h, :],
                            e2[:],
                            start=(pair == 0), stop=(pair == T // 256 - 1),
                            perf_mode=DR)
                    o_t = po_pool.tile([128, 512], BF16, tag="o_t")
                    nc.vector.tensor_copy(o_t[0:65], pv[0:65])
                    rs_t = rsA if h < 4 else rsB
                    rp = 32 * (h % 4)
                    nc.sync.dma_start(rs_t[rp:rp + 1], o_t[64:65])
                    o_ts.append(o_t)
                # batched reciprocal of all 8 rowsums (2 ops), then per-head
                # rank-1 broadcast + normalize
                riA = pb.tile([128, 512], BF16, tag="riA")
                riB = pb.tile([128, 512], BF16, tag="riB")
                rq = pb.tile([128, 2, 512], BF16, tag="rsq")
                nc.vector.tensor_mul(rq[:, 0], rsA[:], rsA[:])
                nc.vector.tensor_mul(rq[:, 1], rsB[:], rsB[:])
                nc.scalar.activation(riA[:], rq[:, 0], AF.Abs_reciprocal_sqrt)
                nc.scalar.activation(riB[:], rq[:, 1], AF.Abs_reciprocal_sqrt)
                for h in range(H):
                    g, par = h // 2, h % 2
                    po = par * 64
                    ri_t = riA if h < 4 else riB
                    rp = 32 * (h % 4)
                    bc = ps_bc2.tile([128, 512], F32, tag="ps_bc")
                    nc.tensor.matmul(
                        bc[0:64],
                        ones_bf[rp:rp + 1, 0:64],
                        ri_t[rp:rp + 1, :],
                        start=True, stop=True, tile_position=(rp, 0))
                    # partition-shifted write is legal (out base 64, 64 rows)
                    nc.vector.tensor_mul(attnT[po:po + 64, g, qs],
                                         o_ts[h][0:64], bc[0:64])

        # ============ phase C: out-proj + residual + LN2 ============
        with tc.tile_pool(name="phC", bufs=2) as pc, \
             tc.tile_pool(name="phDa", bufs=2) as pda, \
             tc.tile_pool(name="psumSt2", bufs=2, space="PSUM") as ps_stat2, \
             tc.tile_pool(name="psumBc3", bufs=2, space="PSUM") as ps_bc3, \
             tc.tile_pool(name="psumC", bufs=2, space="PSUM") as psum:
            gelu_scale = 1.0 / WS if FC1_FP8 else 1.0
            fc2_scale = 1.0 / WS if FC2_FP8 else 1.0
            for qc in range(TQ // 512):
                qs = slice(qc * 512, (qc + 1) * 512)
                # out-proj (fp8 DoubleRow) + residual
                for db in range(4):
                    ps = psum.tile([128, 512], F32, tag="ps_mm")
                    for c2 in range(2):
                        nc.tensor.matmul(
                            ps[:],
                            wout[:, 2 * c2:2 * c2 + 2, db * 128:(db + 1) * 128],
                            attnT[:, 2 * c2:2 * c2 + 2, qs],
                            start=(c2 == 0), stop=(c2 == 1), perf_mode=DR)
                    nc.vector.scalar_tensor_tensor(
                        r1[:, db, qs], ps[:], 1.0 / WS, xt[:, db, qs],
                        op0=ALU.mult, op1=ALU.add)
                    nc.vector.tensor_copy(r1b[:, db, qs], r1[:, db, qs])

                ln_transposed(r1b, 1, qc, ps_stat2, ps_bc3, pc,
                              [r1[:, c] for c in range(4)], [h2])

            # MLP (both qc chunks; LN2 applies overlap the other chunk's mms)
            for qc in range(TQ // 512):
                qs = slice(qc * 512, (qc + 1) * 512)
                a_t = pda.tile([128, 16, 512], a_dt, tag="a_t")
                for jb in range(16):
                    ps = psum.tile([128, 512], F32, tag="ps_mm")
                    if FC1_FP8:
                        for c2 in range(2):
                            nc.tensor.matmul(
                                ps[:],
                                w1[:, 2 * c2:2 * c2 + 2,
                                   jb * 128:(jb + 1) * 128],
                                h2[:, 2 * c2:2 * c2 + 2, qs],
                                start=(c2 == 0), stop=(c2 == 1), perf_mode=DR)
                    else:
                        for c in range(4):
                            nc.tensor.matmul(
                                ps[:], w1[:, c, jb * 128:(jb + 1) * 128],
                                h2[:, c, qs], start=(c == 0), stop=(c == 3))
                    nc.scalar.activation(a_t[:, jb], ps[:], AF.Gelu,
                                         bias=b1t[:, jb:jb + 1],
                                         scale=gelu_scale)
                for db in range(4):
                    ps = psum.tile([128, 512], F32, tag="ps_mm")
                    if FC2_FP8:
                        for j2 in range(8):
                            nc.tensor.matmul(
                                ps[:],
                                w2[:, 2 * j2:2 * j2 + 2,
                                   db * 128:(db + 1) * 128],
                                a_t[:, 2 * j2:2 * j2 + 2],
                                start=(j2 == 0), stop=(j2 == 7), perf_mode=DR)
                    else:
                        for jb in range(16):
                            nc.tensor.matmul(
                                ps[:], w2[:, jb, db * 128:(db + 1) * 128],
                                a_t[:, jb], start=(jb == 0), stop=(jb == 15))
                    outT = pc.tile([128, 512], F32, tag="outT")
                    nc.vector.scalar_tensor_tensor(
                        outT[:], ps[:], fc2_scale, r1[:, db, qs],
                        op0=ALU.mult, op1=ALU.add)
                    nc.sync.dma_start(
                        d_out[db * 128:(db + 1) * 128, qs], outT[:])

        if DEBUG:
            for name, t in [("nT8", nT8), ("nTb", nTb), ("qT", qT),
                            ("kT", kT), ("vnat", vnat), ("attnT", attnT),
                            ("r1", r1), ("h2", h2)]:
                if t is not None:
                    nc.sync.dma_start(d_dbg[name][:], t[:])
    nc.finalize()
    return nc


def kernel(x, ln1_g, ln1_b, w_qkv, w_out, ln2_g, ln2_b, w1, b1, w2, b2):
    x = np.asarray(x, np.float32)
    ln1_g = np.asarray(ln1_g, np.float32); ln1_b = np.asarray(ln1_b, np.float32)
    ln2_g = np.asarray(ln2_g, np.float32); ln2_b = np.asarray(ln2_b, np.float32)
    w_qkv = np.asarray(w_qkv, np.float32); w_out = np.asarray(w_out, np.float32)
    w1 = np.asarray(w1, np.float32); b1 = np.asarray(b1, np.float32)
    w2 = np.asarray(w2, np.float32); b2 = np.asarray(b2, np.float32)

    assert not np.any(ln1_b), "nonzero ln1_b not supported by this kernel"

    # exact-math folds: LN affine params into the adjacent weight matrices
    wqkv_f = ln1_g[:, None] * w_qkv
    w1_f = ln2_g[:, None] * w1
    b1_f = b1 + ln2_b @ w1

    bf = ml_dtypes.bfloat16
    f8 = ml_dtypes.float8_e4m3

    def to_f8(a, scale):
        return np.ascontiguousarray(
            np.clip(a * scale, -240.0, 240.0).astype(f8))

    crep, srep = _rope_tables()

    common = {
        "wqk": to_f8(wqkv_f[:, :2 * D], WS),
        "wv": (to_f8(wqkv_f[:, 2 * D:], WS) if V_FP8
               else np.ascontiguousarray(wqkv_f[:, 2 * D:].astype(bf))),
        "wout": to_f8(w_out, WS),
        "w1": (to_f8(w1_f, WS) if FC1_FP8
               else np.ascontiguousarray(w1_f.astype(bf))),
        "w2": (to_f8(w2, WS) if FC2_FP8
               else np.ascontiguousarray(w2.astype(bf))),
        "b1t": np.ascontiguousarray(
            b1_f.reshape(DFF // 128, 128).T.astype(np.float32)),
    }
    in_maps = []
    for c in range(NC):
        b, half = c // 2, c % 2
        t0 = half * TQ
        rot = np.r_[t0:T, 0:t0]
        m = dict(common)
        m["xt"] = np.ascontiguousarray(x[b].T[:, rot].astype(bf))
        m["cos_k"] = np.ascontiguousarray(crep[:, rot])
        m["sin_k"] = np.ascontiguousarray(srep[:, rot])
        in_maps.append(m)

    if "prog" not in _CACHE:
        _CACHE["prog"] = _build_program()
    nc = _CACHE["prog"]

    _CACHE["in_maps"] = in_maps
    res = run_bass_kernel_spmd(nc, in_maps, core_ids=list(range(NC)))
    out = np.empty((B, T, D), np.float32)
    for c in range(NC):
        b, half = c // 2, c % 2
        out[b, half * TQ:(half + 1) * TQ] = res.results[c]["out"].T
    out += b2[None, None, :]
    return out
